# revision 41
# baseline (speedup 1.0000x reference)
"""BertBlock kernel for 8 Trainium2 NeuronCores.

Sharding: pure data-parallel over (batch, half-sequence) tokens: core c
handles batch element c//2, query-token half c%2 (1024 tokens). Each core
recomputes K/V for the full 2048-token sequence of its batch element (the
duplicated K/V projection work is far cheaper than any 2-rank collective),
so no collectives are needed at all.

Device layout is feature-major ([feature, token]) end to end; the host
pre-transposes the per-core x slices and post-transposes the feature-major
output. Softmax denominators come from an extra ones-column in the
attention-V stationary operand (the PE reduces over keys for free); the
divide is a DMA partition-broadcast of the reciprocal row followed by a
DVE multiply. LayerNorm stats are computed with ones-vector matmuls on the
PE (feature dim lives on partitions) fused into the producing loops, and
the normalize runs in token halves so it overlaps downstream compute.
"""

import numpy as np
import ml_dtypes

P = 128
B = 4
S = 2048          # sequence length (keys)
SQ = 1024         # query tokens per core
HQ = SQ // 2      # token half for LN/MLP pipelining
H = 768
HC = H // P       # 6 feature chunks
NH = 12
DH = 64
FF = 3072
FC = FF // P      # 24
TS = S // P       # 16 key-token chunks
N_CORES = 8
EPS = 1e-5
BF16 = ml_dtypes.bfloat16

_CACHE = {}


def _ln_finish(nc, mybir, pool, bc_pool, ones_row, zero_s, eps_s,
               sum_ap, sq_ap, width, bc_tag, bufs=1):
    """From accumulated sum/sq stats rows covering `width` tokens, produce
    broadcast mean (mb) and rstd (rb) tiles [P, width]. The 1/std uses the
    fast approximate DVE reciprocal (~51 ULP — far below bf16 noise)."""
    f32 = mybir.dt.float32
    f32r = mybir.dt.float32r
    AF = mybir.ActivationFunctionType
    OP = mybir.AluOpType

    mean = pool.tile([1, width], f32r, tag="lnmean", bufs=bufs)
    nc.vector.tensor_scalar_mul(mean[:], sum_ap, 1.0 / H)
    m2 = pool.tile([1, width], f32, tag="lntmp", bufs=2)
    nc.vector.tensor_tensor(m2[:], mean[:], mean[:], OP.mult)
    var = pool.tile([1, width], f32, tag="lntmp", bufs=2)
    nc.vector.scalar_tensor_tensor(
        out=var[:], in0=sq_ap, scalar=1.0 / H, in1=m2[:], op0=OP.mult,
        op1=OP.subtract,
    )
    std = pool.tile([1, width], f32, tag="lntmp", bufs=2)
    nc.scalar.activation(std[:], var[:], AF.Sqrt, bias=eps_s[:])
    rstd_f = pool.tile([1, width], f32, tag="lnrstdf", bufs=bufs)
    nc.vector.reciprocal_approx_fast(rstd_f[:], std[:])
    # round to f32r: the broadcast matmul below requires an f32r producer
    rstd = pool.tile([1, width], f32r, tag="lnrstd", bufs=bufs)
    nc.vector.tensor_copy(rstd[:], rstd_f[:])
    mb_ps = bc_pool.tile([P, width], f32, tag=bc_tag)
    rb_ps = bc_pool.tile([P, width], f32, tag=bc_tag)
    for n in range(0, width, 512):
        sl = slice(n, min(n + 512, width))
        nc.tensor.matmul(mb_ps[:, sl], lhsT=ones_row[:], rhs=mean[:, sl],
                         start=True, stop=True)
        nc.tensor.matmul(rb_ps[:, sl], lhsT=ones_row[:], rhs=rstd[:, sl],
                         start=True, stop=True)
    mb = pool.tile([P, width], f32, tag="lnmb", bufs=bufs)
    nc.scalar.activation(mb[:], mb_ps[:], AF.Identity, bias=zero_s[:])
    rb = pool.tile([P, width], f32, tag="lnrb", bufs=bufs)
    nc.scalar.activation(rb[:], rb_ps[:], AF.Identity, bias=zero_s[:])
    return mb, rb


def _emit(nc, tc, t, mybir):
    """Emit the per-core program. `t` maps tensor name -> DRAM AP."""
    from contextlib import ExitStack

    f32 = mybir.dt.float32
    f32r = mybir.dt.float32r
    bf16 = mybir.dt.bfloat16
    AF = mybir.ActivationFunctionType
    OP = mybir.AluOpType

    def mm(ps, lhsT, rhs, start, stop):
        nc.tensor.matmul(ps, lhsT=lhsT, rhs=rhs, start=start, stop=stop)

    with ExitStack() as ctx:
        aux = ctx.enter_context(tc.tile_pool(name="aux", bufs=1))

        # aux tiles are created up front but their (tiny) DMAs are issued on
        # the sync queue only after the critical x/weight loads, so nothing
        # blocks the first projections
        _aux_loads = []

        def aux_load(name, shape, dtype=f32):
            tl = aux.tile(shape, dtype, tag=name)
            _aux_loads.append((tl, t[name]))
            return tl

        bq_s = aux_load("bq2", [P, HC])
        bk_s = aux_load("bk2", [P, HC])
        bo_s = aux_load("bo2", [P, HC])
        b2_s = aux_load("b22", [P, HC])
        l1w_s = aux_load("l1w", [P, HC])
        l1b_s = aux_load("l1b", [P, HC])
        l2w_s = aux_load("l2w", [P, HC])
        l2b_s = aux_load("l2b", [P, HC])
        b1_s = aux_load("b12", [P, FC])
        bvb_s = aux.tile([P, H], f32)
        _aux_loads.append((bvb_s, t["bv"].partition_broadcast(P)))
        zero_s = aux.tile([P, 1], f32)
        nc.vector.memset(zero_s[:], 0.0)
        eps_s = aux.tile([1, 1], f32)
        nc.vector.memset(eps_s[:], EPS)
        ones_f = aux.tile([P, 1], f32)
        nc.vector.memset(ones_f[:], 1.0)
        ones_s = aux.tile([P, 1], f32r)
        nc.vector.tensor_copy(ones_s[:], ones_f[:])
        ones_rf = aux.tile([1, P], f32)
        nc.vector.memset(ones_rf[:], 1.0)
        ones_row = aux.tile([1, P], f32r)
        nc.vector.tensor_copy(ones_row[:], ones_rf[:])

        # x1 (LN1 output, bf16) outlives the attention/O-proj scopes below.
        keep = ctx.enter_context(tc.tile_pool(name="keep", bufs=1))
        x1b_s = keep.tile([P, HC, SQ], bf16)
        # weight pools live low in SBUF so their DMAs never alias the
        # attention-phase pools and can prefetch during earlier phases.
        # W1 is fully resident (loaded during attention) so MLP1 can run
        # token-half-outer without re-streaming weights.
        wop = ctx.enter_context(tc.tile_pool(name="wo_st", bufs=3))
        w1pool = ctx.enter_context(tc.tile_pool(name="w1_res", bufs=1))
        w1_s = w1pool.tile([P, FC, HC, P], bf16)

        with tc.tile_pool(name="resid", bufs=1) as resid:
            # bf16 query-half slice of x (host-sliced: the half offset
            # differs per core but the program is SPMD). Feeds both the
            # Q-projection and the O-projection residual add; bf16 rounding
            # of the residual costs ~3e-4 relative error.
            xTq_s = resid.tile([P, HC, SQ], bf16)
            with tc.tile_pool(name="attn_out", bufs=1) as aop:
                attnT_s = aop.tile([P, HC, SQ], bf16)

                with tc.tile_pool(name="qkv_keep", bufs=1) as p2:
                    # qTz[p, h, q]: head h's 64 q-rows live at partitions
                    # (h%2)*64..+64 of plane h; the other 64 partitions stay
                    # zero so scores can contract over all 128 partitions.
                    # Only the never-written halves are zeroed, on the DVE
                    # (a gpsimd memset here would block the weight-stream
                    # queue for ~20us).
                    qTz_s = p2.tile([P, NH, SQ], bf16)
                    qTz_v = qTz_s[:].rearrange("p (a two) s -> p a two s", two=2)
                    nc.vector.memset(qTz_v[DH:P, :, 0, :], 0.0)
                    nc.vector.memset(qTz_v[0:DH, :, 1, :], 0.0)
                    kT_s = p2.tile([P, HC, S], bf16)
                    # v_s[p, kt, h*65 .. h*65+64] = V rows for head h,
                    # col h*65+64 = ones (softmax denominator); 63 zero pad
                    # cols at the end let every head take a full 128-col
                    # stationary slice v_s[:, kt, h*65 : h*65+128].
                    v_s = p2.tile([P, TS, NH * (DH + 1) + DH - 1], bf16)
                    v_view = v_s[:, :, 0 : NH * (DH + 1)].rearrange(
                        "p t (h d) -> p t h d", h=NH
                    )
                    nc.vector.memset(v_view[:, :, :, DH : DH + 1], 1.0)
                    nc.vector.memset(v_s[:, :, NH * (DH + 1) :], 0.0)

                    # ---------------- QKV projections ----------------
                    with tc.tile_pool(name="qkvph", bufs=1) as ph, tc.tile_pool(
                        name="wstream", bufs=3
                    ) as ws, tc.tile_pool(
                        name="qkv_ps", bufs=3, space="PSUM"
                    ) as pp:
                        # Q's input loads first (Q is the first projection),
                        # then the full-sequence x for K/V, split into chunks
                        # across two queues.
                        for j in range(HC):
                            eng = nc.sync if j % 2 == 0 else nc.scalar
                            eng.dma_start(
                                xTq_s[:, j, :], t["xTq"][j * P : (j + 1) * P, :]
                            )
                        xT_s = ph.tile([P, HC, S], bf16)
                        for j in range(HC):
                            eng = nc.sync if j % 2 == 0 else nc.scalar
                            eng.dma_start(
                                xT_s[:, j, :], t["xT"][j * P : (j + 1) * P, :]
                            )
                        # aux constants are off the startup critical path
                        for tl, src in _aux_loads:
                            nc.sync.dma_start(tl[:], src)

                        # Q (our 1024 query tokens, bf16 like K)
                        for j in range(HC):
                            w_t = ws.tile([P, HC, P], bf16, tag="w")
                            nc.gpsimd.dma_start(
                                w_t[:],
                                t["Wq"][:, j * P : (j + 1) * P].rearrange(
                                    "(c p) m -> p c m", p=P
                                ),
                            )
                            ps = pp.tile([P, SQ], f32, tag="qkps")
                            for kc in range(HC):
                                for n in range(2):
                                    mm(
                                        ps[:, n * 512 : (n + 1) * 512],
                                        w_t[:, kc, :],
                                        xTq_s[
                                            :, kc, n * 512 : (n + 1) * 512
                                        ],
                                        kc == 0,
                                        kc == HC - 1,
                                    )
                            nc.scalar.activation(
                                qTz_s[0:DH, 2 * j, :], ps[0:DH, :],
                                AF.Identity, bias=bq_s[0:DH, j : j + 1],
                            )
                            nc.scalar.activation(
                                qTz_s[DH:P, 2 * j + 1, :], ps[DH:P, :],
                                AF.Identity, bias=bq_s[DH:P, j : j + 1],
                            )

                        # K (all 2048 tokens, bf16)
                        for j in range(HC):
                            wk_t = ws.tile([P, HC, P], bf16, tag="w")
                            nc.gpsimd.dma_start(
                                wk_t[:],
                                t["Wk"][:, j * P : (j + 1) * P].rearrange(
                                    "(c p) m -> p c m", p=P
                                ),
                            )
                            for hf in range(2):
                                ps = pp.tile([P, SQ], f32, tag="qkps")
                                for kc in range(HC):
                                    for n in range(2):
                                        mm(
                                            ps[:, n * 512 : (n + 1) * 512],
                                            wk_t[:, kc, :],
                                            xT_s[
                                                :, kc,
                                                hf * SQ + n * 512 :
                                                hf * SQ + (n + 1) * 512,
                                            ],
                                            kc == 0,
                                            kc == HC - 1,
                                        )
                                nc.scalar.activation(
                                    kT_s[:, j, hf * SQ : (hf + 1) * SQ],
                                    ps[:],
                                    AF.Identity,
                                    bias=bk_s[:, j : j + 1],
                                )

                        # V (token-major with per-head ones column)
                        wv_t = ws.tile([P, HC, H], bf16, tag="wv", bufs=1)
                        nc.gpsimd.dma_start(
                            wv_t[:], t["Wv"].rearrange("(c p) m -> p c m", p=P)
                        )
                        for tt in range(TS):
                            ps = pp.tile([P, SQ], f32, tag="qkps")
                            for kc in range(HC):
                                mm(
                                    ps[:, 0:512],
                                    xT_s[:, kc, tt * P : (tt + 1) * P],
                                    wv_t[:, kc, 0:512],
                                    kc == 0,
                                    kc == HC - 1,
                                )
                                mm(
                                    ps[:, 512:H],
                                    xT_s[:, kc, tt * P : (tt + 1) * P],
                                    wv_t[:, kc, 512:H],
                                    kc == 0,
                                    kc == HC - 1,
                                )
                            nc.vector.scalar_tensor_tensor(
                                out=v_view[:, tt, :, 0:DH],
                                in0=ps[:, 0:H].rearrange("p (h d) -> p h d", h=NH),
                                scalar=1.0,
                                in1=bvb_s[:].rearrange("p (h d) -> p h d", h=NH),
                                op0=OP.mult,
                                op1=OP.add,
                            )

                    # prefetch W1 during attention on the idle gpsimd queue
                    for m in range(FC):
                        nc.gpsimd.dma_start(
                            w1_s[:, m, :, :],
                            t["W1"][:, m * P : (m + 1) * P].rearrange(
                                "(c p) n -> p c n", p=P
                            ),
                        )

                    # ---------------- attention ----------------
                    with tc.tile_pool(name="attn_sb", bufs=1) as ab, tc.tile_pool(
                        name="probs", bufs=4
                    ) as prp, tc.tile_pool(
                        name="sc_ps", bufs=2, space="PSUM"
                    ) as pps, tc.tile_pool(
                        name="av_ps", bufs=2, space="PSUM"
                    ) as ppa:
                        avs = {}
                        spills = {}

                        def spill_head(h):
                            # Copy the raw accumulator to SBUF right away so
                            # the psum slot frees fast. Runs on the DVE: the
                            # scalar engine is saturated by the softmax exps.
                            # The sums row moves to partition 0 separately —
                            # the custom-DVE reciprocal ucode ignores input
                            # partition offsets on hardware.
                            av = avs.pop(h)
                            avs_sb = ab.tile([DH, SQ], f32, tag="avsb", bufs=2)
                            nc.vector.tensor_copy(avs_sb[:], av[0:DH, :])
                            sums_sb = ab.tile([1, SQ], f32, tag="sums", bufs=2)
                            nc.vector.tensor_copy(
                                sums_sb[:], av[DH : DH + 1, :]
                            )
                            spills[h] = (avs_sb, sums_sb)

                        def normalize_head(h):
                            """Divide head h's attention rows by the softmax
                            sums and place them into attnT.  Emitted one head
                            behind the matmul stream so the PE never waits."""
                            hc = h // 2
                            avs_sb, sums_sb = spills.pop(h)
                            rec_f = ab.tile([1, SQ], f32, tag="recf", bufs=1)
                            nc.vector.reciprocal_approx_fast(
                                rec_f[:], sums_sb[:]
                            )
                            rec = ab.tile([1, SQ], f32r, tag="rec", bufs=1)
                            nc.vector.tensor_copy(rec[:], rec_f[:])
                            bc_ps = ppa.tile([DH, SQ], f32, tag="av")
                            for n in range(2):
                                mm(
                                    bc_ps[:, n * 512 : (n + 1) * 512],
                                    ones_row[:, 0:DH],
                                    rec[:, n * 512 : (n + 1) * 512],
                                    True,
                                    True,
                                )
                            if h % 2 == 0:
                                nc.vector.tensor_tensor(
                                    attnT_s[0:DH, hc, :], avs_sb[:],
                                    bc_ps[:], OP.mult,
                                )
                            else:
                                tmp = ab.tile([DH, SQ], bf16, tag="tmp", bufs=2)
                                nc.vector.tensor_tensor(
                                    tmp[:], avs_sb[:], bc_ps[:], OP.mult
                                )
                                nc.sync.dma_start(
                                    attnT_s[DH:P, hc, :], tmp[:]
                                )

                        def emit_av(h, av, kt, pr):
                            for n in range(2):
                                mm(
                                    av[:, n * 512 : (n + 1) * 512],
                                    v_s[
                                        :, kt,
                                        h * (DH + 1) : h * (DH + 1) + P,
                                    ],
                                    pr[:, n * 512 : (n + 1) * 512],
                                    kt == 0,
                                    kt == TS - 1,
                                )

                        for h in range(NH):
                            hc = h // 2
                            av = ppa.tile([P, SQ], f32, tag="av")
                            avs[h] = av
                            pending = []
                            for kt in range(TS):
                                sc = pps.tile([P, SQ], f32, tag="sc")
                                lhsT_k = kT_s[
                                    :, hc, kt * P : (kt + 1) * P
                                ]
                                for n in range(2):
                                    mm(
                                        sc[:, n * 512 : (n + 1) * 512],
                                        lhsT_k,
                                        qTz_s[
                                            :, h, n * 512 : (n + 1) * 512
                                        ],
                                        True,
                                        True,
                                    )
                                pr = prp.tile([P, SQ], bf16, tag="pr")
                                nc.scalar.activation(
                                    pr[:], sc[:], AF.Exp, bias=zero_s[:],
                                    scale=0.125,
                                )
                                pending.append((kt, pr))
                                if len(pending) > 2:
                                    emit_av(h, av, *pending.pop(0))
                            for p_ in pending:
                                emit_av(h, av, *p_)
                            spill_head(h)
                            if h > 0:
                                normalize_head(h - 1)
                        normalize_head(NH - 1)

                # ------- O-projection + residual + LN1 (stats fused) -------
                with tc.tile_pool(name="oproj", bufs=1) as op_, tc.tile_pool(
                    name="o_ps", bufs=2, space="PSUM"
                ) as ppo, tc.tile_pool(
                    name="st_ps", bufs=1, space="PSUM"
                ) as ppst:
                    r1_s = op_.tile([P, HC, SQ], f32r)
                    sum_ps = ppst.tile([1, SQ], f32, tag="lnsum", bufs=1)
                    sq_ps = ppst.tile([1, SQ], f32, tag="lnsq", bufs=1)

                    def ln1_stats(j, sq_t):
                        for n in range(2):
                            sl = slice(n * 512, (n + 1) * 512)
                            mm(sum_ps[:, sl], ones_s[:], r1_s[:, j, sl],
                               j == 0, j == HC - 1)
                            mm(sq_ps[:, sl], ones_s[:], sq_t[:, sl],
                               j == 0, j == HC - 1)

                    pend_stats = None
                    for j in range(HC):
                        wo_t = wop.tile([P, HC, P], bf16, tag="wo")
                        nc.gpsimd.dma_start(
                            wo_t[:],
                            t["Wo"][:, j * P : (j + 1) * P].rearrange(
                                "(c p) m -> p c m", p=P
                            ),
                        )
                        ps = ppo.tile([P, SQ], f32, tag="ops")
                        for kc in range(HC):
                            for n in range(2):
                                mm(
                                    ps[:, n * 512 : (n + 1) * 512],
                                    wo_t[:, kc, :],
                                    attnT_s[
                                        :, kc, n * 512 : (n + 1) * 512
                                    ],
                                    kc == 0,
                                    kc == HC - 1,
                                )
                        nc.vector.scalar_tensor_tensor(
                            out=r1_s[:, j, :],
                            in0=ps[:],
                            scalar=bo_s[:, j : j + 1],
                            in1=xTq_s[:, j, :],
                            op0=OP.add,
                            op1=OP.add,
                        )
                        sq_t = op_.tile([P, SQ], f32r, tag="lnsqt", bufs=2)
                        nc.vector.tensor_tensor(
                            sq_t[:], r1_s[:, j, :], r1_s[:, j, :], OP.mult
                        )
                        # stats matmuls lag one chunk so the PE never waits
                        # on the DVE mid-loop
                        if pend_stats is not None:
                            ln1_stats(*pend_stats)
                        pend_stats = (j, sq_t)
                    ln1_stats(*pend_stats)
                    mb, rb = _ln_finish(
                        nc, mybir, op_, ppo, ones_row, zero_s, eps_s,
                        sum_ps[:], sq_ps[:], SQ, bc_tag="ops",
                    )
                    # normalize in token halves, writing bf16 x1 directly;
                    # MLP1 (emitted later, half-outer) starts on the first
                    # half while the engines finish the second. Chunks split
                    # across DVE and the idle gpsimd; a bare LDWEIGHTS after
                    # each chunk keeps the PE's HAM clock warm through the
                    # otherwise matmul-free window.
                    for half in range(2):
                        ts_ = slice(half * HQ, (half + 1) * HQ)
                        for j in range(HC):
                            eng = nc.gpsimd if j % 3 == 2 else nc.vector
                            t1 = op_.tile([P, HQ], f32, tag="lnt1", bufs=3)
                            eng.tensor_tensor(
                                t1[:], r1_s[:, j, ts_], mb[:, ts_], OP.subtract
                            )
                            t2 = op_.tile([P, HQ], f32, tag="lnt2", bufs=3)
                            eng.tensor_tensor(t2[:], t1[:], rb[:, ts_], OP.mult)
                            eng.tensor_scalar(
                                x1b_s[:, j, ts_], t2[:], l1w_s[:, j : j + 1],
                                l1b_s[:, j : j + 1], OP.mult, OP.add,
                            )
                            # keep-warm only for half 0: half-1 LDWs would
                            # sit ahead of the ready MLP1-h0 matmuls in the
                            # in-order PE queue and stall them
                            if half == 0:
                                nc.tensor.ldweights(x1b_s[:, j, 0:P])

        # ---------------- MLP + LN2 + output ----------------
        # MLP1 runs token-half-outer (W1 is resident); MLP2+LN2 run in
        # token quarters interleaved with MLP1 so every LayerNorm tail
        # hides behind matmuls of another token range. Order:
        #   MLP1(h0) MLP2(q0) MLP2(q1) MLP1(h1) MLP2(q2) MLP2(q3)
        QW = SQ // 4
        with tc.tile_pool(name="mlp", bufs=1) as mp:
            hT_s = mp.tile([P, FC, SQ], bf16)
            r2_s = mp.tile([P, HC, SQ], f32r)
            # W2 is one large load; issued here so it's resident by MLP2(q0)
            w2_s = mp.tile([P, FC, H], bf16)
            nc.scalar.dma_start(
                w2_s[:], t["W2"].rearrange("(c p) m -> p c m", p=P)
            )
            with tc.tile_pool(
                name="m1_ps", bufs=2, space="PSUM"
            ) as ppm, tc.tile_pool(
                name="m2_ps", bufs=2, space="PSUM"
            ) as ppm2, tc.tile_pool(
                name="st2_ps", bufs=2, space="PSUM"
            ) as ppst2:

                def mlp1_half(half):
                    ts_ = slice(half * HQ, (half + 1) * HQ)
                    for m in range(FC):
                        ps = ppm.tile([P, HQ], f32, tag="mps")
                        for kc in range(HC):
                            mm(
                                ps[:],
                                w1_s[:, m, kc, :],
                                x1b_s[:, kc, ts_],
                                kc == 0,
                                kc == HC - 1,
                            )
                        nc.scalar.activation(
                            hT_s[:, m, ts_], ps[:], AF.Gelu,
                            bias=b1_s[:, m : m + 1],
                        )

                def mlp2_quarter(q):
                    ts_ = slice(q * QW, (q + 1) * QW)
                    sum_ps = ppst2.tile([1, QW], f32, tag="st2sum")
                    sq_ps = ppst2.tile([1, QW], f32, tag="st2sq")

                    def ln2_stats(j, sq_t):
                        mm(sum_ps[:], ones_s[:], r2_s[:, j, ts_],
                           j == 0, j == HC - 1)
                        mm(sq_ps[:], ones_s[:], sq_t[:],
                           j == 0, j == HC - 1)

                    pend = None
                    for j in range(HC):
                        ps = ppm2.tile([P, QW], f32, tag="m2ps")
                        for kc in range(FC):
                            mm(
                                ps[:],
                                w2_s[:, kc, j * P : (j + 1) * P],
                                hT_s[:, kc, ts_],
                                kc == 0,
                                kc == FC - 1,
                            )
                        nc.vector.scalar_tensor_tensor(
                            out=r2_s[:, j, ts_],
                            in0=ps[:],
                            scalar=b2_s[:, j : j + 1],
                            in1=x1b_s[:, j, ts_],
                            op0=OP.add,
                            op1=OP.add,
                        )
                        sq_t = mp.tile([P, QW], f32r, tag="lnsqt2", bufs=2)
                        nc.vector.tensor_tensor(
                            sq_t[:], r2_s[:, j, ts_], r2_s[:, j, ts_], OP.mult
                        )
                        if pend is not None:
                            ln2_stats(*pend)
                        pend = (j, sq_t)
                    ln2_stats(*pend)
                    mb2, rb2 = _ln_finish(
                        nc, mybir, mp, ppm2, ones_row, zero_s, eps_s,
                        sum_ps[:], sq_ps[:], QW, bc_tag="m2ps", bufs=2,
                    )
                    # LN2 normalizes r2 in place, chunks split across DVE
                    # and gpsimd; each chunk is DMA'd straight out (y stays
                    # feature-major — the host does the final transpose).
                    for j in range(HC):
                        eng = nc.gpsimd if j % 3 == 2 else nc.vector
                        t1 = mp.tile([P, QW], f32, tag="lnt1", bufs=3)
                        eng.tensor_tensor(
                            t1[:], r2_s[:, j, ts_], mb2[:], OP.subtract
                        )
                        t2 = mp.tile([P, QW], f32, tag="lnt2", bufs=3)
                        eng.tensor_tensor(t2[:], t1[:], rb2[:], OP.mult)
                        eng.tensor_scalar(
                            r2_s[:, j, ts_], t2[:], l2w_s[:, j : j + 1],
                            l2b_s[:, j : j + 1], OP.mult, OP.add,
                        )
                        deng = nc.sync if j % 2 == 0 else nc.scalar
                        deng.dma_start(
                            t["y"][j * P : (j + 1) * P, ts_],
                            r2_s[:, j, ts_].bitcast(f32),
                        )

                mlp1_half(0)
                mlp2_quarter(0)
                mlp2_quarter(1)
                mlp1_half(1)
                mlp2_quarter(2)
                mlp2_quarter(3)


def _build():
    import concourse.bacc as bacc
    import concourse.tile as tile
    import concourse.mybir as mybir

    f32 = mybir.dt.float32
    f32r = mybir.dt.float32r
    bf16 = mybir.dt.bfloat16

    nc = bacc.Bacc(
        "TRN2", target_bir_lowering=False, debug=False, num_devices=N_CORES
    )
    specs = [
        ("xT", [H, S], bf16, "ExternalInput"),
        ("xTq", [H, SQ], bf16, "ExternalInput"),
        ("Wq", [H, H], bf16, "ExternalInput"),
        ("Wk", [H, H], bf16, "ExternalInput"),
        ("Wv", [H, H], bf16, "ExternalInput"),
        ("Wo", [H, H], bf16, "ExternalInput"),
        ("W1", [H, FF], bf16, "ExternalInput"),
        ("W2", [FF, H], bf16, "ExternalInput"),
        ("bq2", [P, HC], f32, "ExternalInput"),
        ("bk2", [P, HC], f32, "ExternalInput"),
        ("bv", [H], f32, "ExternalInput"),
        ("bo2", [P, HC], f32, "ExternalInput"),
        ("b12", [P, FC], f32, "ExternalInput"),
        ("b22", [P, HC], f32, "ExternalInput"),
        ("l1w", [P, HC], f32, "ExternalInput"),
        ("l1b", [P, HC], f32, "ExternalInput"),
        ("l2w", [P, HC], f32, "ExternalInput"),
        ("l2b", [P, HC], f32, "ExternalInput"),
        ("y", [H, SQ], f32, "ExternalOutput"),
    ]
    t = {
        name: nc.dram_tensor(name, shape, dt, kind=kind).ap()
        for name, shape, dt, kind in specs
    }
    with tile.TileContext(nc) as tc:
        _emit(nc, tc, t, mybir)
    nc.compile()
    return nc


def _chunk_major(v):
    """[C*P] -> [P, C] with entry [p, c] = v[c*P + p]."""
    return np.ascontiguousarray(v.reshape(-1, P).T)


def prepare_in_maps(inputs):
    inp = {k: np.asarray(v) for k, v in inputs.items()}
    x = inp["x"].astype(np.float32)

    shared = {
        "Wq": inp["Wq"].astype(BF16),
        "Wk": inp["Wk"].astype(BF16),
        "Wv": inp["Wv"].astype(BF16),
        "Wo": inp["Wo"].astype(BF16),
        "W1": inp["W1"].astype(BF16),
        "W2": inp["W2"].astype(BF16),
        "bq2": _chunk_major(inp["bq"].astype(np.float32)),
        "bk2": _chunk_major(inp["bk"].astype(np.float32)),
        "bv": inp["bv"].astype(np.float32),
        "bo2": _chunk_major(inp["bo"].astype(np.float32)),
        "b12": _chunk_major(inp["b1"].astype(np.float32)),
        "b22": _chunk_major(inp["b2"].astype(np.float32)),
        "l1w": _chunk_major(inp["ln1_w"].astype(np.float32)),
        "l1b": _chunk_major(inp["ln1_b"].astype(np.float32)),
        "l2w": _chunk_major(inp["ln2_w"].astype(np.float32)),
        "l2b": _chunk_major(inp["ln2_b"].astype(np.float32)),
    }
    in_maps = []
    for c in range(N_CORES):
        b, hf = c // 2, c % 2
        xT = np.ascontiguousarray(x[b].T)
        m = dict(shared)
        m["xT"] = xT.astype(BF16)
        m["xTq"] = np.ascontiguousarray(
            xT[:, hf * SQ : (hf + 1) * SQ]
        ).astype(BF16)
        in_maps.append(m)
    return in_maps


def postprocess(results):
    """Assemble the full [B, S, H] output from per-core feature-major y."""
    out = np.empty((B, S, H), np.float32)
    for c in range(N_CORES):
        b, hf = c // 2, c % 2
        out[b, hf * SQ : (hf + 1) * SQ] = results[c]["y"].T
    return out


def get_program():
    if "nc" not in _CACHE:
        _CACHE["nc"] = _build()
    return _CACHE["nc"]


def kernel(**inputs):
    from concourse.bass_utils import run_bass_kernel_spmd

    nc = get_program()
    in_maps = prepare_in_maps(inputs)
    res = run_bass_kernel_spmd(nc, in_maps, core_ids=list(range(N_CORES)))
    return postprocess(res.results)
